# revision 25
# baseline (speedup 1.0000x reference)
"""Trainium2 Bass kernel for a transformer decoder layer (self-attn + cross-attn + FFN).

Sharding: 8 cores = 4 batches x 2 query-halves. Each core computes K/V for the
full source/target sequence of its batch (duplicated across the 2 cores sharing
a batch) and queries/outputs for its 1024-row half. No cross-core communication.

Layout: activations live TRANSPOSED ([d_model on partitions, tokens free]) so
every linear uses its weight in native [fan_in, fan_out] layout as the
stationary operand. Attention scores are computed transposed ([keys, queries]);
softmax denominators come from one-hot reduction matmuls; normalization is
applied to the attention output before W_o via a broadcast matmul of the
reciprocal row sums. LayerNorm stats use ones-matmul partition reductions and
a broadcast matmul; rsqrt = exp(-0.5*ln(var+eps)) keeps one ACT table set.

fp8: projections (Q/K/V/O), the FFN and the probs@V matmul run in fp8 e4m3
with MatmulPerfMode.DoubleRow (two 128-row contraction tiles per instruction,
2x PE throughput). Weights are pre-scaled by 32 (fan-in 1024) or 64 (fan-in
4096) on the host so U(-1/32,1/32) values escape the e4m3 subnormal range;
evictions descale by the same constant on DVE (fused with the bias add).
Output-projection biases are folded into the residual stream (host-side for
phase A, LayerNorm post-bias for phases B/C). QK^T stays bf16: the score
precision feeds exp() and is the error-critical path.
"""

import os
import sys

import numpy as np

for _p in ("/opt/trn_rl_repo", os.path.expanduser("~/.axon_site/_ro/trn_rl_repo")):
    if os.path.isdir(_p) and _p not in sys.path:
        sys.path.insert(0, _p)

import ml_dtypes  # noqa: E402

import concourse.bass as bass  # noqa: E402
import concourse.tile as tile  # noqa: E402
from concourse import bacc, mybir  # noqa: E402
from concourse.bass_utils import run_bass_kernel_spmd  # noqa: E402

P = 128
D = 1024
H = 16
DK = 64
DFF = 4096
S = 2048          # full sequence (keys)
SL = 1024         # local queries per core
B = 4
DT = D // P       # 8 d-model partition tiles
FT = DFF // P     # 32 ffn partition tiles
SKT = S // P      # 16 key tiles
CH = 256          # LayerNorm column chunk
CW = 512          # attention/FFN query-column chunk
NCW = SL // CW    # 2 chunks
EPS = 1e-5

BF = mybir.dt.bfloat16
F32 = mybir.dt.float32
F8 = mybir.dt.float8e4
AF = mybir.ActivationFunctionType
OP = mybir.AluOpType
DR = mybir.MatmulPerfMode.DoubleRow
BF_NP = ml_dtypes.bfloat16
F8_NP = ml_dtypes.float8_e4m3

_FP8 = os.environ.get("KERNEL_FP8", "proj,ffn,pv")
FP8_PROJ = "proj" in _FP8
FP8_FFN = "ffn" in _FP8
FP8_PV = "pv" in _FP8
WS1 = 32.0        # weight pre-scale for fan-in 1024
WS2 = 64.0        # weight pre-scale for fan-in 4096


def _t(i):
    return slice(i * P, (i + 1) * P)


class _Consts:
    def __init__(self, tc, pool):
        nc = tc.nc
        self.ones_col = pool.tile([P, 1], BF, tag="ones_col")
        nc.vector.memset(self.ones_col[:], 1.0)
        self.ones_row_f = pool.tile([1, P], F32, tag="ones_row_f")
        nc.vector.memset(self.ones_row_f[:], 1.0)
        self.ones_row_b = pool.tile([1, P], BF, tag="ones_row_b")
        nc.vector.memset(self.ones_row_b[:], 1.0)
        self.eps = pool.tile([P, 1], F32, tag="eps")
        nc.vector.memset(self.eps[:], EPS)


class Pools:
    """One SBUF pool + PSUM pools; slot budget is static per (tag, bufs)."""

    def __init__(self, tc, ctx):
        self.tc = tc
        self.sb = ctx.enter_context(tc.tile_pool(name="sb", bufs=1))
        # PSUM budget (8 banks x 2KB): scores 2x2 + pv 2x1 + oproj 1 + gen 1
        self.ps_big = ctx.enter_context(tc.tile_pool(name="ps_big", bufs=2, space="PSUM"))
        self.ps_pv = ctx.enter_context(tc.tile_pool(name="ps_pv", bufs=2, space="PSUM"))
        self.ps_o = ctx.enter_context(tc.tile_pool(name="ps_o", bufs=1, space="PSUM"))
        self.ps_gen = ctx.enter_context(tc.tile_pool(name="ps_gen", bufs=1, space="PSUM"))

    def proj_ps(self, cw=CW):
        # projections borrow a [P, cw]-slice of the big score psum class
        return self.ps_big.tile([P, 2 * CW], F32, tag="scores", name="ps")[:, 0:cw]

    def o_ps(self):
        # O-projection / LN-broadcast bank, decoupled from the score psums so
        # next-chunk scores never wait on this chunk's output projection
        return self.ps_o.tile([P, CW], F32, tag="oproj", name="ps_o")

    def big8(self):
        return self.sb.tile([P, DT, CW], F32, tag="big8", bufs=3, name="big8")

    def b4(self, dt=BF):
        return self.sb.tile([P, DT, CW], dt, tag="b4", bufs=4, name="b4")


def _layernorm_cw(tc, po, consts, x_chunk, out_f, out_b, post_bias=None):
    """LayerNorm over d_model for one [P, DT, CW] f32 chunk (two CH halves).

    The stats of both halves are reduced first so Ln/Exp run once each on a
    [1, CW] vector — one act-table round-trip per CW chunk instead of two.
    out_b: optional low-precision copy (taken BEFORE post_bias); post_bias:
    optional [P, DT] f32 bias folded into out_f only (residual-stream bias of
    the NEXT block's output projection).
    """
    nc = tc.nc
    mu = po.sb.tile([1, CW], F32, tag="ln_mu")
    msq = po.sb.tile([1, CW], F32, tag="ln_msq")
    var = po.sb.tile([1, CW], F32, tag="ln_var")
    rstd = po.sb.tile([1, CW], F32, tag="ln_rstd")
    for lh in range(CW // CH):
        l0 = lh * CH
        cx = po.sb.tile([P, DT, CH], BF, tag="b4", bufs=4, name="lncx")
        sq = po.sb.tile([P, DT, CH], BF, tag="b4", bufs=4, name="lnsq")
        for t in range(DT):
            nc.vector.tensor_copy(cx[:, t, :], x_chunk[:, t, l0 : l0 + CH])
            nc.vector.tensor_tensor(
                sq[:, t, :], x_chunk[:, t, l0 : l0 + CH],
                x_chunk[:, t, l0 : l0 + CH], OP.mult,
            )
        pstat = po.ps_gen.tile([P, 2 * CH], F32, tag="gen")
        for kt in range(DT):
            nc.tensor.matmul(
                pstat[0:1, 0:CH], consts.ones_col[:], cx[:, kt, :],
                start=(kt == 0), stop=(kt == DT - 1), tile_position=(0, 0),
                skip_group_check=True,
            )
            nc.tensor.matmul(
                pstat[32:33, 0:CH], consts.ones_col[:], sq[:, kt, :],
                start=(kt == 0), stop=(kt == DT - 1), tile_position=(0, 32),
                skip_group_check=True,
            )
        nc.scalar.mul(mu[:, l0 : l0 + CH], pstat[0:1, 0:CH], 1.0 / D)
        nc.scalar.mul(msq[:, l0 : l0 + CH], pstat[32:33, 0:CH], 1.0 / D)
    nc.vector.tensor_tensor(var[:], mu[:], mu[:], OP.mult)
    nc.vector.tensor_sub(var[:], msq[:], var[:])
    nc.scalar.activation(var[:], var[:], AF.Ln, bias=consts.eps[0:1, :])
    nc.scalar.activation(rstd[:], var[:], AF.Exp, scale=-0.5)
    for lh in range(CW // CH):
        l0 = lh * CH
        pb = po.o_ps()
        nc.tensor.matmul(
            pb[:, 0:CH], consts.ones_row_f[:], mu[:, l0 : l0 + CH],
            start=True, stop=False,
        )
        nc.tensor.matmul(
            pb[:, CH : 2 * CH], consts.ones_row_f[:], rstd[:, l0 : l0 + CH],
            start=False, stop=True,
        )
        for t in range(DT):
            nc.vector.tensor_tensor(
                out_f[:, t, l0 : l0 + CH], x_chunk[:, t, l0 : l0 + CH],
                pb[:, 0:CH], OP.subtract,
            )
            nc.vector.tensor_tensor(
                out_f[:, t, l0 : l0 + CH], out_f[:, t, l0 : l0 + CH],
                pb[:, CH : 2 * CH], OP.mult,
            )
            if out_b is not None:
                nc.vector.tensor_copy(out_b[:, t, l0 : l0 + CH], out_f[:, t, l0 : l0 + CH])
            if post_bias is not None:
                nc.vector.tensor_scalar_add(
                    out_f[:, t, l0 : l0 + CH], out_f[:, t, l0 : l0 + CH],
                    post_bias[:, t : t + 1],
                )


def _attention_chunk(tc, po, consts, KT, Vaug, qt_c, attn_c):
    """One query chunk (CW columns) of MHA in transposed layout, per head.

    KT: [P, DT, S] bf16; Vaug: [P, SKT, H, DK+1] (natural V per head with a
    ones column appended -> PV matmuls emit the softmax denominator in row 64);
    qt_c: [P, DT, CW] bf16 (pre-scaled by 1/8); attn_c: [P, DT, CW] out.
    """
    nc = tc.nc
    e_dt = F8 if FP8_PV else BF
    for h in range(H):
        hp, prow = h // 2, (h % 2) * DK
        comb = po.ps_pv.tile([P, CW], F32, tag="pv", name="comb")
        for sp in range(SKT // 2):  # pairs of key tiles
            k0 = 2 * sp
            ps_s = po.ps_big.tile([P, 2 * CW], F32, tag="scores", name="ps_s")
            for qi in range(2):
                # each half is a full PSUM bank: start zeroes its own region
                nc.tensor.matmul(
                    ps_s[:, qi * CW : (qi + 1) * CW],
                    KT[prow : prow + DK, hp, _t(k0 + qi)],
                    qt_c[prow : prow + DK, hp, :],
                    start=True, stop=True,
                )
            e = po.sb.tile([P, 2 * CW], e_dt, tag="exp", bufs=2, name="e")
            nc.scalar.activation(e[:], ps_s[:], AF.Exp)
            if FP8_PV:
                nc.tensor.matmul(
                    comb[0 : DK + 1, :],
                    Vaug[:, k0 : k0 + 2, h, :],
                    e[:].rearrange("p (two n) -> p two n", two=2),
                    start=(sp == 0), stop=(sp == SKT // 2 - 1),
                    perf_mode=DR,
                )
            else:
                for qi in range(2):
                    nc.tensor.matmul(
                        comb[0 : DK + 1, :],
                        Vaug[:, k0 + qi, h, :],
                        e[:, qi * CW : (qi + 1) * CW],
                        start=(sp == 0 and qi == 0),
                        stop=(sp == SKT // 2 - 1 and qi == 1),
                    )
        # normalize: the rowsum sits in row DK of the accumulator
        rf0 = po.sb.tile([1, CW], F32, tag="rf0", bufs=2, name="rf0")
        nc.vector.reciprocal(rf0[:], comb[DK : DK + 1, :])
        rfb = po.sb.tile([1, CW], BF, tag="rfb", bufs=2, name="rfb")
        nc.vector.tensor_copy(rfb[:], rf0[:])
        ps_r = po.ps_gen.tile([P, CW], F32, tag="gen", name="ps_r")
        nc.tensor.matmul(
            ps_r[0:DK, :], consts.ones_row_b[:, 0:DK], rfb[:], start=True, stop=True
        )
        rbc = po.sb.tile([DK, CW], BF, tag="rbc", bufs=2, name="rbc")
        nc.vector.tensor_copy(rbc[:], ps_r[0:DK, :])
        nc.vector.tensor_tensor(
            attn_c[prow : prow + DK, hp, :], comb[0:DK, :], rbc[:], OP.mult
        )


def build_program():
    nc = bacc.Bacc("TRN2", target_bir_lowering=False, debug=False, num_devices=8)

    act_dt = F8 if FP8_PROJ else BF
    ffn_dt = F8 if FP8_FFN else BF

    def din(name, shape, dt=BF):
        return nc.dram_tensor(name, list(shape), dt, kind="ExternalInput").ap()

    tgtT = din("tgtT", (D, S), act_dt)
    tgtLocT = din("tgtLocT", (D, SL), F32)
    srcT = din("srcT", (D, S), act_dt)
    w = {}
    for pre in ("sa", "ca"):
        for nm in ("wq", "wk", "wv", "wo"):
            w[f"{pre}_{nm}"] = din(f"{pre}_{nm}", (D, D), act_dt)
        w[f"{pre}_bqT"] = din(f"{pre}_bqT", (P, DT), F32)
        w[f"{pre}_bkT"] = din(f"{pre}_bkT", (P, DT), F32)
        w[f"{pre}_bvB"] = din(f"{pre}_bvB", (P, D), BF)
    w["ca_boT"] = din("ca_boT", (P, DT), F32)
    w["ff_w1"] = din("ff_w1", (D, DFF), ffn_dt)
    w["ff_w2"] = din("ff_w2", (DFF, D), ffn_dt)
    w["ff_b1T"] = din("ff_b1T", (P, FT), F32)
    w["ff_b2T"] = din("ff_b2T", (P, DT), F32)

    outT = nc.dram_tensor("outT", [D, SL], F32, kind="ExternalOutput").ap()
    x1f = nc.dram_tensor("x1f", [D, SL], F32).ap()
    x1b = nc.dram_tensor("x1b", [D, SL], act_dt).ap()
    x2f = nc.dram_tensor("x2f", [D, SL], F32).ap()

    def r3(ap):  # [(t p), s] dram -> [p, t, s]
        return ap.rearrange("(t p) s -> p t s", p=P)

    import contextlib

    reps = int(os.environ.get("KERNEL_REPS", "1"))
    with tile.TileContext(nc) as tc, contextlib.ExitStack() as ctx:
        po = Pools(tc, ctx)
        consts = _Consts(tc, po.sb)

        def load_w_block(dram_ap, t_n, cols, dt=BF):
            t_ = po.sb.tile([P, t_n, 1024], dt, tag="w", bufs=2, name="wblk")[:, :, : cols.stop - cols.start]
            nc.sync.dma_start(t_[:], r3(dram_ap)[:, :t_n, cols])
            return t_

        def load_bias(name, n):
            t_ = po.sb.tile([P, FT], F32, tag=f"b_{name}", name=f"b_{name}")[:, :n]
            nc.sync.dma_start(t_[:], w[name][:, :n])
            return t_

        def proj_T(w_sb, rhs_fn, evict_fn, n_cols, out_tiles=DT, cw=CW, dr=False):
            for t_out in range(out_tiles):
                for c0 in range(0, n_cols, cw):
                    pt = po.proj_ps(cw)
                    if dr:
                        for kt in range(0, DT, 2):
                            nc.tensor.matmul(
                                pt[:], w_sb[:, kt : kt + 2, _t(t_out)],
                                rhs_fn(kt, c0),
                                start=(kt == 0), stop=(kt == DT - 2),
                                perf_mode=DR,
                            )
                    else:
                        for kt in range(DT):
                            nc.tensor.matmul(
                                pt[:], w_sb[:, kt, _t(t_out)], rhs_fn(kt, c0),
                                start=(kt == 0), stop=(kt == DT - 1),
                            )
                    evict_fn(t_out, c0, pt)

        def attn_phase(pre, kv_srcT, q_loader, resid_f, x_out_f, x_out_b,
                       x_out_b_sb=None, post_bias=None, qw=CH):
            KT = po.sb.tile([P, DT, S], BF, tag="KT")
            v_dt = F8 if FP8_PV else BF
            Vaug = po.sb.tile([P, SKT, H, DK + 1], v_dt, tag="Vn")
            nc.vector.memset(Vaug[:, :, :, DK : DK + 1], 1.0)
            wk = load_w_block(w[f"{pre}_wk"], DT, slice(0, D), dt=act_dt)
            bkT = load_bias(f"{pre}_bkT", DT)
            wsc = 1.0 / WS1 if FP8_PROJ else None

            KW = 512

            def k_evict(t, c0, pt):
                if FP8_PROJ:
                    nc.vector.tensor_scalar(
                        KT[:, t, c0 : c0 + KW], pt[:], wsc,
                        bkT[:, t : t + 1], OP.mult, OP.add,
                    )
                else:
                    nc.vector.tensor_scalar_add(
                        KT[:, t, c0 : c0 + KW], pt[:], bkT[:, t : t + 1]
                    )

            proj_T(
                wk,
                lambda kt, c0: (
                    kv_srcT[:, kt : kt + 2, c0 : c0 + KW] if FP8_PROJ
                    else kv_srcT[:, kt, c0 : c0 + KW]
                ),
                k_evict, S, cw=KW, dr=FP8_PROJ,
            )
            wv = load_w_block(w[f"{pre}_wv"], DT, slice(0, D), dt=act_dt)
            bvB = po.sb.tile([P, D], BF, tag="bvB", bufs=1)
            nc.sync.dma_start(bvB[:], w[f"{pre}_bvB"][:])
            VW = 512
            HPC = VW // DK  # heads per column chunk
            for skt in range(SKT):
                for dc in range(D // VW):
                    pt = po.proj_ps(VW)
                    if FP8_PROJ:
                        for kt in range(0, DT, 2):
                            nc.tensor.matmul(
                                pt[:], kv_srcT[:, kt : kt + 2, _t(skt)],
                                wv[:, kt : kt + 2, dc * VW : (dc + 1) * VW],
                                start=(kt == 0), stop=(kt == DT - 2),
                                perf_mode=DR,
                            )
                    else:
                        for kt in range(DT):
                            nc.tensor.matmul(
                                pt[:], kv_srcT[:, kt, _t(skt)],
                                wv[:, kt, dc * VW : (dc + 1) * VW],
                                start=(kt == 0), stop=(kt == DT - 1),
                            )
                    if FP8_PROJ:
                        nc.vector.scalar_tensor_tensor(
                            Vaug[:, skt, dc * HPC : (dc + 1) * HPC, 0:DK],
                            pt[:].rearrange("p (a b) -> p a b", a=HPC),
                            wsc,
                            bvB[:, dc * VW : (dc + 1) * VW].rearrange(
                                "p (a b) -> p a b", a=HPC
                            ),
                            OP.mult, OP.add,
                        )
                    else:
                        nc.vector.tensor_tensor(
                            Vaug[:, skt, dc * HPC : (dc + 1) * HPC, 0:DK],
                            pt[:].rearrange("p (a b) -> p a b", a=HPC),
                            bvB[:, dc * VW : (dc + 1) * VW].rearrange(
                                "p (a b) -> p a b", a=HPC
                            ),
                            OP.add,
                        )
            wq = load_w_block(w[f"{pre}_wq"], DT, slice(0, D), dt=act_dt)
            bqT = load_bias(f"{pre}_bqT", DT)  # pre-scaled by 1/8 on host
            wo = load_w_block(w[f"{pre}_wo"], DT, slice(0, D), dt=act_dt)
            qsc = 0.125 / WS1 if FP8_PROJ else 0.125

            def q_evict(t, c0, pt, dest):
                nc.vector.tensor_scalar(
                    dest, pt[:], qsc, bqT[:, t : t + 1], OP.mult, OP.add
                )

            # project Q for ALL chunks up-front (frees kv/q sources early and
            # lets the attention chunks pipeline back-to-back)
            qt_all = po.sb.tile([P, DT, SL], BF, tag="qtA", name="qt_all")
            for b0 in range(0, SL, qw):
                q_src = q_loader(b0)
                proj_T(
                    wq,
                    lambda kt, c0, q_src=q_src: q_src(kt, c0),
                    lambda t, c0, pt, b0=b0: q_evict(
                        t, c0, pt, qt_all[:, t, b0 + c0 : b0 + c0 + 512]
                    ),
                    qw, cw=512, dr=FP8_PROJ,
                )
            for c in range(NCW):
                c0 = c * CW
                attn_c = po.b4(act_dt)
                _attention_chunk(
                    tc, po, consts, KT, Vaug, qt_all[:, :, c0 : c0 + CW], attn_c
                )
                x_chunk = po.big8()
                for t_out in range(DT):
                    pt = po.o_ps()
                    if FP8_PROJ:
                        for kt in range(0, DT, 2):
                            nc.tensor.matmul(
                                pt[:], wo[:, kt : kt + 2, _t(t_out)],
                                attn_c[:, kt : kt + 2, :],
                                start=(kt == 0), stop=(kt == DT - 2),
                                perf_mode=DR,
                            )
                        nc.vector.scalar_tensor_tensor(
                            x_chunk[:, t_out, :], pt[:], wsc,
                            resid_f(t_out, c0), OP.mult, OP.add,
                        )
                    else:
                        for kt in range(DT):
                            nc.tensor.matmul(
                                pt[:], wo[:, kt, _t(t_out)], attn_c[:, kt, :],
                                start=(kt == 0), stop=(kt == DT - 1),
                            )
                        nc.vector.tensor_tensor(
                            x_chunk[:, t_out, :], pt[:], resid_f(t_out, c0), OP.add
                        )
                xnf = po.big8()
                xnb = (
                    x_out_b_sb[:, :, c0 : c0 + CW]
                    if x_out_b_sb is not None
                    else po.b4(act_dt)
                )
                _layernorm_cw(tc, po, consts, x_chunk, xnf, xnb, post_bias)
                nc.sync.dma_start(r3(x_out_f)[:, :, c0 : c0 + CW], xnf[:])
                if x_out_b_sb is None:
                    nc.sync.dma_start(r3(x_out_b)[:, :, c0 : c0 + CW], xnb[:])

        phases = os.environ.get("KERNEL_PHASES", "abc")
        for _rep in range(reps):
            # ---- Phase A: self-attention on tgt ----
            tgtT_sb = po.sb.tile([P, DT, S], act_dt, tag="actT", name="tgtT_sb")
            for s0 in range(0, S, 512):  # column-split so K-proj starts early
                nc.sync.dma_start(
                    tgtT_sb[:, :, s0 : s0 + 512], r3(tgtT)[:, :, s0 : s0 + 512]
                )

            def tgt_resid(t, c0):
                # tgtLocT has sa_bo pre-added on the host
                rt = po.sb.tile([P, CW], F32, tag="resid", bufs=2, name="resid")
                nc.sync.dma_start(rt[:], r3(tgtLocT)[:, t, c0 : c0 + CW])
                return rt[:]

            ca_boT = load_bias("ca_boT", DT)
            attn_phase(
                "sa", tgtT_sb,
                lambda b0: (lambda kt, c0: (
                    tgtT_sb[:, kt : kt + 2, b0 + c0 : b0 + c0 + 512] if FP8_PROJ
                    else tgtT_sb[:, kt, b0 + c0 : b0 + c0 + 512]
                )),
                tgt_resid, x1f, x1b, post_bias=ca_boT, qw=SL,
            )

            if "b" not in phases:
                continue
            # ---- Phase B: cross-attention ----
            srcT_sb = po.sb.tile([P, DT, S], act_dt, tag="actT", name="srcT_sb")
            for s0 in range(0, S, 512):
                nc.sync.dma_start(
                    srcT_sb[:, :, s0 : s0 + 512], r3(srcT)[:, :, s0 : s0 + 512]
                )

            def x1_qsrc(b0):
                qt = po.sb.tile([P, DT, SL], act_dt, tag="big8", bufs=3, name="qsrc")
                nc.sync.dma_start(qt[:], r3(x1b)[:, :, b0 : b0 + SL])
                return lambda kt, c0: (
                    qt[:, kt : kt + 2, c0 : c0 + 512] if FP8_PROJ
                    else qt[:, kt, c0 : c0 + 512]
                )

            def x1_resid(t, c0):
                # x1f has ca_bo folded in by phase A's LayerNorm post-bias
                rt = po.sb.tile([P, CW], F32, tag="resid", bufs=2, name="resid")
                nc.sync.dma_start(rt[:], r3(x1f)[:, t, c0 : c0 + CW])
                return rt[:]

            # LN2's low-precision output stays in SBUF for phase C (actT slot
            # is free once the ca K/V projections have consumed srcT_sb)
            ff_b2T = load_bias("ff_b2T", DT)
            x2n_all = po.sb.tile([P, DT, SL], ffn_dt, tag="actT", name="x2n_all")
            attn_phase("ca", srcT_sb, x1_qsrc, x1_resid, x2f, None,
                       x_out_b_sb=x2n_all, post_bias=ff_b2T, qw=SL)

            if "c" not in phases:
                continue
            # ---- Phase C: FFN, d_ff quarters OUTER so each weight block is
            # loaded once; partial sums for all chunks accumulate in SBUF ----
            b1T = load_bias("ff_b1T", FT)
            QF = 1024 // P  # ff-tiles per quarter
            acc_all = po.sb.tile([P, DT, SL], F32, tag="KT", name="acc_all")
            w2sc = 1.0 / WS2 if FP8_FFN else None
            for qtr in range(4):
                w1q = load_w_block(w["ff_w1"], DT, slice(qtr * 1024, (qtr + 1) * 1024), dt=ffn_dt)
                w2q = po.sb.tile([P, QF, D], ffn_dt, tag="w", bufs=2, name="w2q")
                nc.sync.dma_start(
                    w2q[:], r3(w["ff_w2"])[:, qtr * QF : (qtr + 1) * QF, :]
                )
                for c in range(NCW):
                    c0 = c * CW
                    hq = po.sb.tile([P, QF, CW], ffn_dt, tag="b4", bufs=4, name="hq")
                    for fo in range(QF):
                        ft = qtr * QF + fo
                        pt = po.proj_ps()
                        if FP8_FFN:
                            for kt in range(0, DT, 2):
                                nc.tensor.matmul(
                                    pt[:], w1q[:, kt : kt + 2, _t(fo)],
                                    x2n_all[:, kt : kt + 2, c0 : c0 + CW],
                                    start=(kt == 0), stop=(kt == DT - 2),
                                    perf_mode=DR,
                                )
                            nc.scalar.activation(
                                hq[:, fo, :], pt[:], AF.Relu,
                                bias=b1T[:, ft : ft + 1], scale=1.0 / WS1,
                            )
                        else:
                            for kt in range(DT):
                                nc.tensor.matmul(
                                    pt[:], w1q[:, kt, _t(fo)],
                                    x2n_all[:, kt, c0 : c0 + CW],
                                    start=(kt == 0), stop=(kt == DT - 1),
                                )
                            nc.scalar.activation(
                                hq[:, fo, :], pt[:], AF.Relu, bias=b1T[:, ft : ft + 1]
                            )
                    for t_out in range(DT):
                        pt = po.proj_ps()
                        if FP8_FFN:
                            for fo in range(0, QF, 2):
                                nc.tensor.matmul(
                                    pt[:], w2q[:, fo : fo + 2, _t(t_out)],
                                    hq[:, fo : fo + 2, :],
                                    start=(fo == 0), stop=(fo == QF - 2),
                                    perf_mode=DR,
                                )
                            if qtr == 0:
                                nc.vector.tensor_scalar_mul(
                                    acc_all[:, t_out, c0 : c0 + CW], pt[:], w2sc
                                )
                            else:
                                nc.vector.scalar_tensor_tensor(
                                    acc_all[:, t_out, c0 : c0 + CW], pt[:], w2sc,
                                    acc_all[:, t_out, c0 : c0 + CW], OP.mult, OP.add,
                                )
                        else:
                            for fo in range(QF):
                                nc.tensor.matmul(
                                    pt[:], w2q[:, fo, _t(t_out)], hq[:, fo, :],
                                    start=(fo == 0), stop=(fo == QF - 1),
                                )
                            if qtr == 0:
                                nc.vector.tensor_copy(acc_all[:, t_out, c0 : c0 + CW], pt[:])
                            else:
                                nc.vector.tensor_tensor(
                                    acc_all[:, t_out, c0 : c0 + CW],
                                    acc_all[:, t_out, c0 : c0 + CW], pt[:], OP.add,
                                )
                    if qtr == 3:
                        # chunk c is fully accumulated: residual + LN now, so
                        # the LN chains overlap chunk c+1's matmuls
                        x3_chunk = po.big8()
                        for t_out in range(DT):
                            # x2f has ff_b2 folded in by phase B's LN post-bias
                            rt = po.sb.tile([P, CW], F32, tag="resid", bufs=2, name="resid")
                            nc.sync.dma_start(rt[:], r3(x2f)[:, t_out, c0 : c0 + CW])
                            nc.vector.tensor_tensor(
                                x3_chunk[:, t_out, :], acc_all[:, t_out, c0 : c0 + CW],
                                rt[:], OP.add,
                            )
                        out_f = po.big8()
                        _layernorm_cw(tc, po, consts, x3_chunk, out_f, None)
                        nc.sync.dma_start(r3(outT)[:, :, c0 : c0 + CW], out_f[:])

    nc.compile()
    return nc


_NC_CACHE = {}


def _get_nc():
    if "nc" not in _NC_CACHE:
        _NC_CACHE["nc"] = build_program()
    return _NC_CACHE["nc"]


def make_in_maps(inputs):
    tgt = np.asarray(inputs["tgt"], np.float32)
    src = np.asarray(inputs["src"], np.float32)
    act_np = F8_NP if FP8_PROJ else BF_NP
    ffn_np = F8_NP if FP8_FFN else BF_NP
    wsc = WS1 if FP8_PROJ else 1.0

    shared = {}
    for pre in ("sa", "ca"):
        for nm in ("wq", "wk", "wv", "wo"):
            shared[f"{pre}_{nm}"] = np.ascontiguousarray(
                (np.asarray(inputs[f"{pre}_{nm}"], np.float32) * wsc).astype(act_np)
            )
        bq = np.asarray(inputs[f"{pre}_bq"], np.float32) * 0.125
        shared[f"{pre}_bqT"] = np.ascontiguousarray(bq.reshape(DT, P).T)
        shared[f"{pre}_bkT"] = np.ascontiguousarray(
            np.asarray(inputs[f"{pre}_bk"], np.float32).reshape(DT, P).T
        )
        shared[f"{pre}_bvB"] = np.ascontiguousarray(
            np.broadcast_to(np.asarray(inputs[f"{pre}_bv"]), (P, D)).astype(BF_NP)
        )
    shared["ca_boT"] = np.ascontiguousarray(
        np.asarray(inputs["ca_bo"], np.float32).reshape(DT, P).T
    )
    w2sc = WS2 if FP8_FFN else 1.0
    shared["ff_w1"] = np.ascontiguousarray(
        (np.asarray(inputs["ff_w1"], np.float32) * (WS1 if FP8_FFN else 1.0)).astype(ffn_np)
    )
    shared["ff_w2"] = np.ascontiguousarray(
        (np.asarray(inputs["ff_w2"], np.float32) * w2sc).astype(ffn_np)
    )
    shared["ff_b1T"] = np.ascontiguousarray(
        np.asarray(inputs["ff_b1"], np.float32).reshape(FT, P).T
    )
    shared["ff_b2T"] = np.ascontiguousarray(
        np.asarray(inputs["ff_b2"], np.float32).reshape(DT, P).T
    )
    sa_bo = np.asarray(inputs["sa_bo"], np.float32)

    in_maps = []
    for core in range(8):
        b, q = core // 2, core % 2
        tT = np.ascontiguousarray(tgt[b].T)  # [D, S] f32
        if q == 1:
            tT = np.concatenate([tT[:, SL:], tT[:, :SL]], axis=1)
        m = dict(shared)
        m["tgtT"] = np.ascontiguousarray(tT.astype(act_np))
        m["tgtLocT"] = np.ascontiguousarray(tT[:, :SL] + sa_bo[:, None])
        m["srcT"] = np.ascontiguousarray(src[b].T.astype(act_np))
        in_maps.append(m)
    return in_maps


def assemble_output(results):
    out = np.empty((B, S, D), np.float32)
    for core in range(8):
        b, q = core // 2, core % 2
        out[b, q * SL : (q + 1) * SL, :] = results[core]["outT"].T
    return out


def kernel(**inputs):
    nc = _get_nc()
    in_maps = make_in_maps(inputs)
    res = run_bass_kernel_spmd(nc, in_maps, list(range(8)))
    return assemble_output(res.results)


if __name__ == "__main__":
    nc = build_program()
    print("program built + compiled OK")


# revision 30
# speedup vs baseline: 1.1470x; 1.1470x over previous
"""Trainium2 Bass kernel for a transformer decoder layer (self-attn + cross-attn + FFN).

Sharding: 8 cores = 4 batches x 2 query-halves. Each core computes K/V for the
full source/target sequence of its batch (duplicated across the 2 cores sharing
a batch) and queries/outputs for its 1024-row half. No cross-core communication.

Layout: activations live TRANSPOSED ([d_model on partitions, tokens free]) so
every linear uses its weight in native [fan_in, fan_out] layout as the
stationary operand. Attention scores are computed transposed ([keys, queries]);
softmax denominators come from one-hot reduction matmuls; normalization is
applied to the attention output before W_o via a broadcast matmul of the
reciprocal row sums. LayerNorm stats use ones-matmul partition reductions and
a broadcast matmul; rsqrt = exp(-0.5*ln(var+eps)) keeps one ACT table set.

fp8: projections (Q/K/V/O), the FFN and the probs@V matmul run in fp8 e4m3
with MatmulPerfMode.DoubleRow (two 128-row contraction tiles per instruction,
2x PE throughput). Weights are pre-scaled by 32 (fan-in 1024) or 64 (fan-in
4096) on the host so U(-1/32,1/32) values escape the e4m3 subnormal range;
evictions descale by the same constant on DVE (fused with the bias add).
Output-projection biases are folded into the residual stream (host-side for
phase A, LayerNorm post-bias for phases B/C). QK^T stays bf16: the score
precision feeds exp() and is the error-critical path.
"""

import os
import sys

import numpy as np

for _p in ("/opt/trn_rl_repo", os.path.expanduser("~/.axon_site/_ro/trn_rl_repo")):
    if os.path.isdir(_p) and _p not in sys.path:
        sys.path.insert(0, _p)

import ml_dtypes  # noqa: E402

import concourse.bass as bass  # noqa: E402
import concourse.tile as tile  # noqa: E402
from concourse import bacc, mybir  # noqa: E402
from concourse.bass_utils import run_bass_kernel_spmd  # noqa: E402

P = 128
D = 1024
H = 16
DK = 64
DFF = 4096
S = 2048          # full sequence (keys)
SL = 1024         # local queries per core
B = 4
DT = D // P       # 8 d-model partition tiles
FT = DFF // P     # 32 ffn partition tiles
SKT = S // P      # 16 key tiles
CH = 256          # LayerNorm column chunk
CW = 512          # attention/FFN query-column chunk
NCW = SL // CW    # 2 chunks
EPS = 1e-5

BF = mybir.dt.bfloat16
F32 = mybir.dt.float32
F8 = mybir.dt.float8e4
AF = mybir.ActivationFunctionType
OP = mybir.AluOpType
DR = mybir.MatmulPerfMode.DoubleRow
BF_NP = ml_dtypes.bfloat16
F8_NP = ml_dtypes.float8_e4m3

_FP8 = os.environ.get("KERNEL_FP8", "proj,ffn,pv")
FP8_PROJ = "proj" in _FP8
FP8_FFN = "ffn" in _FP8
FP8_PV = "pv" in _FP8
WS1 = 32.0        # weight pre-scale for fan-in 1024
WS2 = 64.0        # weight pre-scale for fan-in 4096


def _t(i):
    return slice(i * P, (i + 1) * P)


class _Consts:
    def __init__(self, tc, pool):
        nc = tc.nc
        self.ones_col = pool.tile([P, 1], BF, tag="ones_col")
        nc.vector.memset(self.ones_col[:], 1.0)
        self.ones_row_f = pool.tile([1, P], F32, tag="ones_row_f")
        nc.vector.memset(self.ones_row_f[:], 1.0)
        self.ones_row_b = pool.tile([1, P], BF, tag="ones_row_b")
        nc.vector.memset(self.ones_row_b[:], 1.0)
        self.eps = pool.tile([P, 1], F32, tag="eps")
        nc.vector.memset(self.eps[:], EPS)


class Pools:
    """One SBUF pool + PSUM pools; slot budget is static per (tag, bufs)."""

    def __init__(self, tc, ctx):
        self.tc = tc
        self.sb = ctx.enter_context(tc.tile_pool(name="sb", bufs=1))
        # PSUM budget (8 banks x 2KB): scores 2x2 + pv 2x1 + oproj 1 + gen 1
        self.ps_big = ctx.enter_context(tc.tile_pool(name="ps_big", bufs=2, space="PSUM"))
        self.ps_pv = ctx.enter_context(tc.tile_pool(name="ps_pv", bufs=2, space="PSUM"))
        self.ps_o = ctx.enter_context(tc.tile_pool(name="ps_o", bufs=1, space="PSUM"))
        self.ps_gen = ctx.enter_context(tc.tile_pool(name="ps_gen", bufs=1, space="PSUM"))

    def proj_ps(self, cw=CW):
        # projections borrow a [P, cw]-slice of the big score psum class
        return self.ps_big.tile([P, 2 * CW], F32, tag="scores", name="ps")[:, 0:cw]

    def o_ps(self):
        # O-projection / LN-broadcast bank, decoupled from the score psums so
        # next-chunk scores never wait on this chunk's output projection
        return self.ps_o.tile([P, CW], F32, tag="oproj", name="ps_o")

    def big8(self):
        return self.sb.tile([P, DT, CW], F32, tag="big8", bufs=3, name="big8")

    def b4(self, dt=BF):
        return self.sb.tile([P, DT, CW], dt, tag="b4", bufs=4, name="b4")


def _layernorm_cw(tc, po, consts, x_chunk, out_f, out_b, post_bias=None):
    """LayerNorm over d_model for one [P, DT, CW] f32 chunk (two CH halves).

    The stats of both halves are reduced first so Ln/Exp run once each on a
    [1, CW] vector — one act-table round-trip per CW chunk instead of two.
    out_b: optional low-precision copy (taken BEFORE post_bias); post_bias:
    optional [P, DT] f32 bias folded into out_f only (residual-stream bias of
    the NEXT block's output projection).
    """
    nc = tc.nc
    mu = po.sb.tile([1, CW], F32, tag="ln_mu")
    msq = po.sb.tile([1, CW], F32, tag="ln_msq")
    var = po.sb.tile([1, CW], F32, tag="ln_var")
    rstd = po.sb.tile([1, CW], F32, tag="ln_rstd")
    for lh in range(CW // CH):
        l0 = lh * CH
        cx = po.sb.tile([P, DT, CH], BF, tag="b4", bufs=4, name="lncx")
        sq = po.sb.tile([P, DT, CH], BF, tag="b4", bufs=4, name="lnsq")
        xs = x_chunk[:, :, l0 : l0 + CH]
        nc.vector.tensor_copy(cx[:], xs)
        nc.vector.tensor_tensor(sq[:], xs, xs, OP.mult)
        pstat = po.ps_gen.tile([P, 2 * CH], F32, tag="gen")
        for kt in range(DT):
            nc.tensor.matmul(
                pstat[0:1, 0:CH], consts.ones_col[:], cx[:, kt, :],
                start=(kt == 0), stop=(kt == DT - 1), tile_position=(0, 0),
                skip_group_check=True,
            )
            nc.tensor.matmul(
                pstat[32:33, 0:CH], consts.ones_col[:], sq[:, kt, :],
                start=(kt == 0), stop=(kt == DT - 1), tile_position=(0, 32),
                skip_group_check=True,
            )
        nc.scalar.mul(mu[:, l0 : l0 + CH], pstat[0:1, 0:CH], 1.0 / D)
        nc.scalar.mul(msq[:, l0 : l0 + CH], pstat[32:33, 0:CH], 1.0 / D)
    nc.vector.tensor_tensor(var[:], mu[:], mu[:], OP.mult)
    nc.vector.tensor_sub(var[:], msq[:], var[:])
    nc.scalar.activation(var[:], var[:], AF.Ln, bias=consts.eps[0:1, :])
    nc.scalar.activation(rstd[:], var[:], AF.Exp, scale=-0.5)
    for lh in range(CW // CH):
        l0 = lh * CH
        pb = po.o_ps()
        nc.tensor.matmul(
            pb[:, 0:CH], consts.ones_row_f[:], mu[:, l0 : l0 + CH],
            start=True, stop=False,
        )
        nc.tensor.matmul(
            pb[:, CH : 2 * CH], consts.ones_row_f[:], rstd[:, l0 : l0 + CH],
            start=False, stop=True,
        )
        xs = x_chunk[:, :, l0 : l0 + CH]
        of = out_f[:, :, l0 : l0 + CH]
        mu_b = pb[:, 0:CH].rearrange("p (o n) -> p o n", o=1).to_broadcast((P, DT, CH))
        rs_b = pb[:, CH : 2 * CH].rearrange("p (o n) -> p o n", o=1).to_broadcast((P, DT, CH))
        nc.vector.tensor_tensor(of, xs, mu_b, OP.subtract)
        nc.vector.tensor_tensor(of, of, rs_b, OP.mult)
        if out_b is not None:
            nc.vector.tensor_copy(out_b[:, :, l0 : l0 + CH], of)
        if post_bias is not None:
            pb_b = post_bias.rearrange("p (t o) -> p t o", o=1).to_broadcast((P, DT, CH))
            nc.vector.tensor_tensor(of, of, pb_b, OP.add)


def _attention_chunk(tc, po, consts, KT, Vaug, qt_c, attn_c):
    """One query chunk (CW columns) of MHA in transposed layout, per head.

    KT: [P, DT, S] bf16; Vaug: [P, SKT, H, DK+1] (natural V per head with a
    ones column appended -> PV matmuls emit the softmax denominator in row 64);
    qt_c: [P, DT, CW] bf16 (pre-scaled by 1/8); attn_c: [P, DT, CW] out.
    """
    nc = tc.nc
    e_dt = F8 if FP8_PV else BF
    for h in range(H):
        hp, prow = h // 2, (h % 2) * DK
        comb = po.ps_pv.tile([P, CW], F32, tag="pv", name="comb")
        for sp in range(SKT // 2):  # pairs of key tiles
            k0 = 2 * sp
            ps_s = po.ps_big.tile([P, 2 * CW], F32, tag="scores", name="ps_s")
            for qi in range(2):
                # each half is a full PSUM bank: start zeroes its own region
                nc.tensor.matmul(
                    ps_s[:, qi * CW : (qi + 1) * CW],
                    KT[prow : prow + DK, hp, _t(k0 + qi)],
                    qt_c[prow : prow + DK, hp, :],
                    start=True, stop=True,
                )
            e = po.sb.tile([P, 2 * CW], e_dt, tag="exp", bufs=2, name="e")
            nc.scalar.activation(e[:], ps_s[:], AF.Exp)
            if FP8_PV:
                nc.tensor.matmul(
                    comb[0 : DK + 1, :],
                    Vaug[:, k0 : k0 + 2, h, :],
                    e[:].rearrange("p (two n) -> p two n", two=2),
                    start=(sp == 0), stop=(sp == SKT // 2 - 1),
                    perf_mode=DR,
                )
            else:
                for qi in range(2):
                    nc.tensor.matmul(
                        comb[0 : DK + 1, :],
                        Vaug[:, k0 + qi, h, :],
                        e[:, qi * CW : (qi + 1) * CW],
                        start=(sp == 0 and qi == 0),
                        stop=(sp == SKT // 2 - 1 and qi == 1),
                    )
        # normalize: the rowsum sits in row DK of the accumulator
        rf0 = po.sb.tile([1, CW], F32, tag="rf0", bufs=2, name="rf0")
        nc.vector.reciprocal(rf0[:], comb[DK : DK + 1, :])
        rfb = po.sb.tile([1, CW], BF, tag="rfb", bufs=2, name="rfb")
        nc.vector.tensor_copy(rfb[:], rf0[:])
        ps_r = po.ps_gen.tile([P, CW], F32, tag="gen", name="ps_r")
        nc.tensor.matmul(
            ps_r[0:DK, :], consts.ones_row_b[:, 0:DK], rfb[:], start=True, stop=True
        )
        rbc = po.sb.tile([DK, CW], BF, tag="rbc", bufs=2, name="rbc")
        nc.vector.tensor_copy(rbc[:], ps_r[0:DK, :])
        nc.vector.tensor_tensor(
            attn_c[prow : prow + DK, hp, :], comb[0:DK, :], rbc[:], OP.mult
        )


def build_program():
    nc = bacc.Bacc("TRN2", target_bir_lowering=False, debug=False, num_devices=8)

    act_dt = F8 if FP8_PROJ else BF
    ffn_dt = F8 if FP8_FFN else BF

    def din(name, shape, dt=BF):
        return nc.dram_tensor(name, list(shape), dt, kind="ExternalInput").ap()

    tgtT = din("tgtT", (D, S), act_dt)
    tgtLocT = din("tgtLocT", (D, SL), F32)
    srcT = din("srcT", (D, S), act_dt)
    w = {}
    for pre in ("sa", "ca"):
        for nm in ("wq", "wk", "wv", "wo"):
            w[f"{pre}_{nm}"] = din(f"{pre}_{nm}", (D, D), act_dt)
        w[f"{pre}_bqT"] = din(f"{pre}_bqT", (P, DT), F32)
        w[f"{pre}_bkT"] = din(f"{pre}_bkT", (P, DT), F32)
        w[f"{pre}_bvB"] = din(f"{pre}_bvB", (P, D), BF)
    w["ca_boT"] = din("ca_boT", (P, DT), F32)
    w["ff_w1"] = din("ff_w1", (D, DFF), ffn_dt)
    w["ff_w2"] = din("ff_w2", (DFF, D), ffn_dt)
    w["ff_b1T"] = din("ff_b1T", (P, FT), F32)
    w["ff_b2T"] = din("ff_b2T", (P, DT), F32)

    outT = nc.dram_tensor("outT", [D, SL], F32, kind="ExternalOutput").ap()
    x1f = nc.dram_tensor("x1f", [D, SL], F32).ap()
    x1b = nc.dram_tensor("x1b", [D, SL], act_dt).ap()
    x2f = nc.dram_tensor("x2f", [D, SL], F32).ap()

    def r3(ap):  # [(t p), s] dram -> [p, t, s]
        return ap.rearrange("(t p) s -> p t s", p=P)

    import contextlib

    reps = int(os.environ.get("KERNEL_REPS", "1"))
    with tile.TileContext(nc) as tc, contextlib.ExitStack() as ctx:
        po = Pools(tc, ctx)
        consts = _Consts(tc, po.sb)

        def load_w_block(dram_ap, t_n, cols, dt=BF):
            t_ = po.sb.tile([P, t_n, 1024], dt, tag="w", bufs=2, name="wblk")[:, :, : cols.stop - cols.start]
            nc.sync.dma_start(t_[:], r3(dram_ap)[:, :t_n, cols])
            return t_

        def load_bias(name, n):
            t_ = po.sb.tile([P, FT], F32, tag=f"b_{name}", name=f"b_{name}")[:, :n]
            nc.sync.dma_start(t_[:], w[name][:, :n])
            return t_

        def proj_T(w_sb, rhs_fn, evict_fn, n_cols, out_tiles=DT, cw=CW, dr=False):
            for t_out in range(out_tiles):
                for c0 in range(0, n_cols, cw):
                    pt = po.proj_ps(cw)
                    if dr:
                        for kt in range(0, DT, 2):
                            nc.tensor.matmul(
                                pt[:], w_sb[:, kt : kt + 2, _t(t_out)],
                                rhs_fn(kt, c0),
                                start=(kt == 0), stop=(kt == DT - 2),
                                perf_mode=DR,
                            )
                    else:
                        for kt in range(DT):
                            nc.tensor.matmul(
                                pt[:], w_sb[:, kt, _t(t_out)], rhs_fn(kt, c0),
                                start=(kt == 0), stop=(kt == DT - 1),
                            )
                    evict_fn(t_out, c0, pt)

        def attn_phase(pre, kv_srcT, q_loader, resid_f, x_out_f, x_out_b,
                       x_out_b_sb=None, post_bias=None, qw=CH):
            KT = po.sb.tile([P, DT, S], BF, tag="KT")
            v_dt = F8 if FP8_PV else BF
            Vaug = po.sb.tile([P, SKT, H, DK + 1], v_dt, tag="Vn")
            nc.vector.memset(Vaug[:, :, :, DK : DK + 1], 1.0)
            wk = load_w_block(w[f"{pre}_wk"], DT, slice(0, D), dt=act_dt)
            bkT = load_bias(f"{pre}_bkT", DT)
            wsc = 1.0 / WS1 if FP8_PROJ else None

            KW = 512

            def k_evict(t, c0, pt):
                if FP8_PROJ:
                    nc.vector.tensor_scalar(
                        KT[:, t, c0 : c0 + KW], pt[:], wsc,
                        bkT[:, t : t + 1], OP.mult, OP.add,
                    )
                else:
                    nc.vector.tensor_scalar_add(
                        KT[:, t, c0 : c0 + KW], pt[:], bkT[:, t : t + 1]
                    )

            proj_T(
                wk,
                lambda kt, c0: (
                    kv_srcT[:, kt : kt + 2, c0 : c0 + KW] if FP8_PROJ
                    else kv_srcT[:, kt, c0 : c0 + KW]
                ),
                k_evict, S, cw=KW, dr=FP8_PROJ,
            )
            wv = load_w_block(w[f"{pre}_wv"], DT, slice(0, D), dt=act_dt)
            bvB = po.sb.tile([P, D], BF, tag="bvB", bufs=1)
            nc.sync.dma_start(bvB[:], w[f"{pre}_bvB"][:])
            VW = 512
            HPC = VW // DK  # heads per column chunk
            for skt in range(SKT):
                for dc in range(D // VW):
                    pt = po.proj_ps(VW)
                    if FP8_PROJ:
                        for kt in range(0, DT, 2):
                            nc.tensor.matmul(
                                pt[:], kv_srcT[:, kt : kt + 2, _t(skt)],
                                wv[:, kt : kt + 2, dc * VW : (dc + 1) * VW],
                                start=(kt == 0), stop=(kt == DT - 2),
                                perf_mode=DR,
                            )
                    else:
                        for kt in range(DT):
                            nc.tensor.matmul(
                                pt[:], kv_srcT[:, kt, _t(skt)],
                                wv[:, kt, dc * VW : (dc + 1) * VW],
                                start=(kt == 0), stop=(kt == DT - 1),
                            )
                    if FP8_PROJ:
                        nc.vector.scalar_tensor_tensor(
                            Vaug[:, skt, dc * HPC : (dc + 1) * HPC, 0:DK],
                            pt[:].rearrange("p (a b) -> p a b", a=HPC),
                            wsc,
                            bvB[:, dc * VW : (dc + 1) * VW].rearrange(
                                "p (a b) -> p a b", a=HPC
                            ),
                            OP.mult, OP.add,
                        )
                    else:
                        nc.vector.tensor_tensor(
                            Vaug[:, skt, dc * HPC : (dc + 1) * HPC, 0:DK],
                            pt[:].rearrange("p (a b) -> p a b", a=HPC),
                            bvB[:, dc * VW : (dc + 1) * VW].rearrange(
                                "p (a b) -> p a b", a=HPC
                            ),
                            OP.add,
                        )
            wq = load_w_block(w[f"{pre}_wq"], DT, slice(0, D), dt=act_dt)
            bqT = load_bias(f"{pre}_bqT", DT)  # pre-scaled by 1/8 on host
            wo = load_w_block(w[f"{pre}_wo"], DT, slice(0, D), dt=act_dt)
            qsc = 0.125 / WS1 if FP8_PROJ else 0.125

            def q_evict(t, c0, pt, dest):
                nc.vector.tensor_scalar(
                    dest, pt[:], qsc, bqT[:, t : t + 1], OP.mult, OP.add
                )

            # project Q for ALL chunks up-front (frees kv/q sources early and
            # lets the attention chunks pipeline back-to-back)
            qt_all = po.sb.tile([P, DT, SL], BF, tag="qtA", name="qt_all")
            for b0 in range(0, SL, qw):
                q_src = q_loader(b0)
                proj_T(
                    wq,
                    lambda kt, c0, q_src=q_src: q_src(kt, c0),
                    lambda t, c0, pt, b0=b0: q_evict(
                        t, c0, pt, qt_all[:, t, b0 + c0 : b0 + c0 + 512]
                    ),
                    qw, cw=512, dr=FP8_PROJ,
                )
            for c in range(NCW):
                c0 = c * CW
                attn_c = po.b4(act_dt)
                _attention_chunk(
                    tc, po, consts, KT, Vaug, qt_all[:, :, c0 : c0 + CW], attn_c
                )
                x_chunk = po.big8()
                rt3 = resid_f(c0)
                for t_out in range(DT):
                    pt = po.o_ps()
                    if FP8_PROJ:
                        for kt in range(0, DT, 2):
                            nc.tensor.matmul(
                                pt[:], wo[:, kt : kt + 2, _t(t_out)],
                                attn_c[:, kt : kt + 2, :],
                                start=(kt == 0), stop=(kt == DT - 2),
                                perf_mode=DR,
                            )
                        nc.vector.scalar_tensor_tensor(
                            x_chunk[:, t_out, :], pt[:], wsc,
                            rt3[:, t_out, :], OP.mult, OP.add,
                        )
                    else:
                        for kt in range(DT):
                            nc.tensor.matmul(
                                pt[:], wo[:, kt, _t(t_out)], attn_c[:, kt, :],
                                start=(kt == 0), stop=(kt == DT - 1),
                            )
                        nc.vector.tensor_tensor(
                            x_chunk[:, t_out, :], pt[:], rt3[:, t_out, :], OP.add
                        )
                xnf = po.big8()
                xnb = (
                    x_out_b_sb[:, :, c0 : c0 + CW]
                    if x_out_b_sb is not None
                    else po.b4(act_dt)
                )
                _layernorm_cw(tc, po, consts, x_chunk, xnf, xnb, post_bias)
                nc.sync.dma_start(r3(x_out_f)[:, :, c0 : c0 + CW], xnf[:])
                if x_out_b_sb is None:
                    nc.sync.dma_start(r3(x_out_b)[:, :, c0 : c0 + CW], xnb[:])

        phases = os.environ.get("KERNEL_PHASES", "abc")
        for _rep in range(reps):
            # ---- Phase A: self-attention on tgt ----
            tgtT_sb = po.sb.tile([P, DT, S], act_dt, tag="actT", name="tgtT_sb")
            for s0 in range(0, S, 512):  # column-split so K-proj starts early
                nc.sync.dma_start(
                    tgtT_sb[:, :, s0 : s0 + 512], r3(tgtT)[:, :, s0 : s0 + 512]
                )

            def tgt_resid(c0):
                # tgtLocT has sa_bo pre-added on the host
                rt = po.sb.tile([P, DT, CW], F32, tag="resid", bufs=1, name="resid")
                nc.sync.dma_start(rt[:], r3(tgtLocT)[:, :, c0 : c0 + CW])
                return rt[:]

            ca_boT = load_bias("ca_boT", DT)
            attn_phase(
                "sa", tgtT_sb,
                lambda b0: (lambda kt, c0: (
                    tgtT_sb[:, kt : kt + 2, b0 + c0 : b0 + c0 + 512] if FP8_PROJ
                    else tgtT_sb[:, kt, b0 + c0 : b0 + c0 + 512]
                )),
                tgt_resid, x1f, x1b, post_bias=ca_boT, qw=SL,
            )

            if "b" not in phases:
                continue
            # ---- Phase B: cross-attention ----
            srcT_sb = po.sb.tile([P, DT, S], act_dt, tag="actT", name="srcT_sb")
            for s0 in range(0, S, 512):
                nc.sync.dma_start(
                    srcT_sb[:, :, s0 : s0 + 512], r3(srcT)[:, :, s0 : s0 + 512]
                )

            def x1_qsrc(b0):
                qt = po.sb.tile([P, DT, SL], act_dt, tag="big8", bufs=3, name="qsrc")
                nc.sync.dma_start(qt[:], r3(x1b)[:, :, b0 : b0 + SL])
                return lambda kt, c0: (
                    qt[:, kt : kt + 2, c0 : c0 + 512] if FP8_PROJ
                    else qt[:, kt, c0 : c0 + 512]
                )

            def x1_resid(c0):
                # x1f has ca_bo folded in by phase A's LayerNorm post-bias
                rt = po.sb.tile([P, DT, CW], F32, tag="resid", bufs=1, name="resid")
                nc.sync.dma_start(rt[:], r3(x1f)[:, :, c0 : c0 + CW])
                return rt[:]

            # LN2's low-precision output stays in SBUF for phase C (actT slot
            # is free once the ca K/V projections have consumed srcT_sb)
            ff_b2T = load_bias("ff_b2T", DT)
            x2n_all = po.sb.tile([P, DT, SL], ffn_dt, tag="actT", name="x2n_all")
            attn_phase("ca", srcT_sb, x1_qsrc, x1_resid, x2f, None,
                       x_out_b_sb=x2n_all, post_bias=ff_b2T, qw=SL)

            if "c" not in phases:
                continue
            # ---- Phase C: FFN, d_ff quarters OUTER so each weight block is
            # loaded once; partial sums for all chunks accumulate in SBUF ----
            b1T = load_bias("ff_b1T", FT)
            QF = 1024 // P  # ff-tiles per quarter
            acc_all = po.sb.tile([P, DT, SL], F32, tag="KT", name="acc_all")
            w2sc = 1.0 / WS2 if FP8_FFN else None
            for qtr in range(4):
                w1q = load_w_block(w["ff_w1"], DT, slice(qtr * 1024, (qtr + 1) * 1024), dt=ffn_dt)
                w2q = po.sb.tile([P, QF, D], ffn_dt, tag="w", bufs=2, name="w2q")
                nc.sync.dma_start(
                    w2q[:], r3(w["ff_w2"])[:, qtr * QF : (qtr + 1) * QF, :]
                )
                for c in range(NCW):
                    c0 = c * CW
                    hq = po.sb.tile([P, QF, CW], ffn_dt, tag="b4", bufs=4, name="hq")
                    for fo in range(QF):
                        ft = qtr * QF + fo
                        pt = po.proj_ps()
                        if FP8_FFN:
                            for kt in range(0, DT, 2):
                                nc.tensor.matmul(
                                    pt[:], w1q[:, kt : kt + 2, _t(fo)],
                                    x2n_all[:, kt : kt + 2, c0 : c0 + CW],
                                    start=(kt == 0), stop=(kt == DT - 2),
                                    perf_mode=DR,
                                )
                            nc.scalar.activation(
                                hq[:, fo, :], pt[:], AF.Relu,
                                bias=b1T[:, ft : ft + 1], scale=1.0 / WS1,
                            )
                        else:
                            for kt in range(DT):
                                nc.tensor.matmul(
                                    pt[:], w1q[:, kt, _t(fo)],
                                    x2n_all[:, kt, c0 : c0 + CW],
                                    start=(kt == 0), stop=(kt == DT - 1),
                                )
                            nc.scalar.activation(
                                hq[:, fo, :], pt[:], AF.Relu, bias=b1T[:, ft : ft + 1]
                            )
                    for t_out in range(DT):
                        pt = po.proj_ps()
                        if FP8_FFN:
                            for fo in range(0, QF, 2):
                                nc.tensor.matmul(
                                    pt[:], w2q[:, fo : fo + 2, _t(t_out)],
                                    hq[:, fo : fo + 2, :],
                                    start=(fo == 0), stop=(fo == QF - 2),
                                    perf_mode=DR,
                                )
                            if qtr == 0:
                                nc.vector.tensor_scalar_mul(
                                    acc_all[:, t_out, c0 : c0 + CW], pt[:], w2sc
                                )
                            else:
                                nc.vector.scalar_tensor_tensor(
                                    acc_all[:, t_out, c0 : c0 + CW], pt[:], w2sc,
                                    acc_all[:, t_out, c0 : c0 + CW], OP.mult, OP.add,
                                )
                        else:
                            for fo in range(QF):
                                nc.tensor.matmul(
                                    pt[:], w2q[:, fo, _t(t_out)], hq[:, fo, :],
                                    start=(fo == 0), stop=(fo == QF - 1),
                                )
                            if qtr == 0:
                                nc.vector.tensor_copy(acc_all[:, t_out, c0 : c0 + CW], pt[:])
                            else:
                                nc.vector.tensor_tensor(
                                    acc_all[:, t_out, c0 : c0 + CW],
                                    acc_all[:, t_out, c0 : c0 + CW], pt[:], OP.add,
                                )
                    if qtr == 3:
                        # chunk c is fully accumulated: residual + LN now, so
                        # the LN chains overlap chunk c+1's matmuls
                        x3_chunk = po.big8()
                        # x2f has ff_b2 folded in by phase B's LN post-bias
                        rt3 = po.sb.tile([P, DT, CW], F32, tag="resid", bufs=1, name="resid")
                        nc.sync.dma_start(rt3[:], r3(x2f)[:, :, c0 : c0 + CW])
                        nc.vector.tensor_tensor(
                            x3_chunk[:], acc_all[:, :, c0 : c0 + CW], rt3[:], OP.add,
                        )
                        out_f = po.big8()
                        _layernorm_cw(tc, po, consts, x3_chunk, out_f, None)
                        nc.sync.dma_start(r3(outT)[:, :, c0 : c0 + CW], out_f[:])

    nc.compile()
    return nc


_NC_CACHE = {}


def _get_nc():
    if "nc" not in _NC_CACHE:
        _NC_CACHE["nc"] = build_program()
    return _NC_CACHE["nc"]


def make_in_maps(inputs):
    tgt = np.asarray(inputs["tgt"], np.float32)
    src = np.asarray(inputs["src"], np.float32)
    act_np = F8_NP if FP8_PROJ else BF_NP
    ffn_np = F8_NP if FP8_FFN else BF_NP
    wsc = WS1 if FP8_PROJ else 1.0

    shared = {}
    for pre in ("sa", "ca"):
        for nm in ("wq", "wk", "wv", "wo"):
            shared[f"{pre}_{nm}"] = np.ascontiguousarray(
                (np.asarray(inputs[f"{pre}_{nm}"], np.float32) * wsc).astype(act_np)
            )
        bq = np.asarray(inputs[f"{pre}_bq"], np.float32) * 0.125
        shared[f"{pre}_bqT"] = np.ascontiguousarray(bq.reshape(DT, P).T)
        shared[f"{pre}_bkT"] = np.ascontiguousarray(
            np.asarray(inputs[f"{pre}_bk"], np.float32).reshape(DT, P).T
        )
        shared[f"{pre}_bvB"] = np.ascontiguousarray(
            np.broadcast_to(np.asarray(inputs[f"{pre}_bv"]), (P, D)).astype(BF_NP)
        )
    shared["ca_boT"] = np.ascontiguousarray(
        np.asarray(inputs["ca_bo"], np.float32).reshape(DT, P).T
    )
    w2sc = WS2 if FP8_FFN else 1.0
    shared["ff_w1"] = np.ascontiguousarray(
        (np.asarray(inputs["ff_w1"], np.float32) * (WS1 if FP8_FFN else 1.0)).astype(ffn_np)
    )
    shared["ff_w2"] = np.ascontiguousarray(
        (np.asarray(inputs["ff_w2"], np.float32) * w2sc).astype(ffn_np)
    )
    shared["ff_b1T"] = np.ascontiguousarray(
        np.asarray(inputs["ff_b1"], np.float32).reshape(FT, P).T
    )
    shared["ff_b2T"] = np.ascontiguousarray(
        np.asarray(inputs["ff_b2"], np.float32).reshape(DT, P).T
    )
    sa_bo = np.asarray(inputs["sa_bo"], np.float32)

    in_maps = []
    for core in range(8):
        b, q = core // 2, core % 2
        tT = np.ascontiguousarray(tgt[b].T)  # [D, S] f32
        if q == 1:
            tT = np.concatenate([tT[:, SL:], tT[:, :SL]], axis=1)
        m = dict(shared)
        m["tgtT"] = np.ascontiguousarray(tT.astype(act_np))
        m["tgtLocT"] = np.ascontiguousarray(tT[:, :SL] + sa_bo[:, None])
        m["srcT"] = np.ascontiguousarray(src[b].T.astype(act_np))
        in_maps.append(m)
    return in_maps


def assemble_output(results):
    out = np.empty((B, S, D), np.float32)
    for core in range(8):
        b, q = core // 2, core % 2
        out[b, q * SL : (q + 1) * SL, :] = results[core]["outT"].T
    return out


def kernel(**inputs):
    nc = _get_nc()
    in_maps = make_in_maps(inputs)
    res = run_bass_kernel_spmd(nc, in_maps, list(range(8)))
    return assemble_output(res.results)


if __name__ == "__main__":
    nc = build_program()
    print("program built + compiled OK")


# revision 31
# speedup vs baseline: 1.3017x; 1.1349x over previous
"""Trainium2 Bass kernel for a transformer decoder layer (self-attn + cross-attn + FFN).

Sharding: 8 cores = 4 batches x 2 query-halves. Each core computes K/V for the
full source/target sequence of its batch (duplicated across the 2 cores sharing
a batch) and queries/outputs for its 1024-row half. No cross-core communication.

Layout: activations live TRANSPOSED ([d_model on partitions, tokens free]) so
every linear uses its weight in native [fan_in, fan_out] layout as the
stationary operand. Attention scores are computed transposed ([keys, queries]);
softmax denominators come from one-hot reduction matmuls; normalization is
applied to the attention output before W_o via a broadcast matmul of the
reciprocal row sums. LayerNorm stats use ones-matmul partition reductions and
a broadcast matmul; rsqrt = exp(-0.5*ln(var+eps)) keeps one ACT table set.

fp8: projections (Q/K/V/O), the FFN and the probs@V matmul run in fp8 e4m3
with MatmulPerfMode.DoubleRow (two 128-row contraction tiles per instruction,
2x PE throughput). Weights are pre-scaled by 32 (fan-in 1024) or 64 (fan-in
4096) on the host so U(-1/32,1/32) values escape the e4m3 subnormal range;
evictions descale by the same constant on DVE (fused with the bias add).
Output-projection biases are folded into the residual stream (host-side for
phase A, LayerNorm post-bias for phases B/C). QK^T stays bf16: the score
precision feeds exp() and is the error-critical path.
"""

import os
import sys

import numpy as np

for _p in ("/opt/trn_rl_repo", os.path.expanduser("~/.axon_site/_ro/trn_rl_repo")):
    if os.path.isdir(_p) and _p not in sys.path:
        sys.path.insert(0, _p)

import ml_dtypes  # noqa: E402

import concourse.bass as bass  # noqa: E402
import concourse.tile as tile  # noqa: E402
from concourse import bacc, mybir  # noqa: E402
from concourse.bass_utils import run_bass_kernel_spmd  # noqa: E402

P = 128
D = 1024
H = 16
DK = 64
DFF = 4096
S = 2048          # full sequence (keys)
SL = 1024         # local queries per core
B = 4
DT = D // P       # 8 d-model partition tiles
FT = DFF // P     # 32 ffn partition tiles
SKT = S // P      # 16 key tiles
CH = 256          # LayerNorm column chunk
CW = 512          # attention/FFN query-column chunk
NCW = SL // CW    # 2 chunks
EPS = 1e-5

BF = mybir.dt.bfloat16
F32 = mybir.dt.float32
F8 = mybir.dt.float8e4
AF = mybir.ActivationFunctionType
OP = mybir.AluOpType
DR = mybir.MatmulPerfMode.DoubleRow
BF_NP = ml_dtypes.bfloat16
F8_NP = ml_dtypes.float8_e4m3

_FP8 = os.environ.get("KERNEL_FP8", "proj,ffn,pv")
FP8_PROJ = "proj" in _FP8
FP8_FFN = "ffn" in _FP8
FP8_PV = "pv" in _FP8
WS1 = 32.0        # weight pre-scale for fan-in 1024
WS2 = 64.0        # weight pre-scale for fan-in 4096


def _t(i):
    return slice(i * P, (i + 1) * P)


class _Consts:
    def __init__(self, tc, pool):
        nc = tc.nc
        self.ones_col = pool.tile([P, 1], BF, tag="ones_col")
        nc.vector.memset(self.ones_col[:], 1.0)
        self.ones_row_f = pool.tile([1, P], F32, tag="ones_row_f")
        nc.vector.memset(self.ones_row_f[:], 1.0)
        self.ones_row_b = pool.tile([1, P], BF, tag="ones_row_b")
        nc.vector.memset(self.ones_row_b[:], 1.0)
        self.eps = pool.tile([P, 1], F32, tag="eps")
        nc.vector.memset(self.eps[:], EPS)


class Pools:
    """One SBUF pool + PSUM pools; slot budget is static per (tag, bufs)."""

    def __init__(self, tc, ctx):
        self.tc = tc
        self.sb = ctx.enter_context(tc.tile_pool(name="sb", bufs=1))
        # PSUM budget (8 banks x 2KB): scores 2x2 + pv 2x1 + oproj 1 + gen 1
        self.ps_big = ctx.enter_context(tc.tile_pool(name="ps_big", bufs=2, space="PSUM"))
        self.ps_pv = ctx.enter_context(tc.tile_pool(name="ps_pv", bufs=2, space="PSUM"))
        self.ps_o = ctx.enter_context(tc.tile_pool(name="ps_o", bufs=1, space="PSUM"))
        self.ps_gen = ctx.enter_context(tc.tile_pool(name="ps_gen", bufs=1, space="PSUM"))

    def proj_ps(self, cw=CW):
        # projections borrow a [P, cw]-slice of the big score psum class
        return self.ps_big.tile([P, 2 * CW], F32, tag="scores", name="ps")[:, 0:cw]

    def o_ps(self):
        # O-projection / LN-broadcast bank, decoupled from the score psums so
        # next-chunk scores never wait on this chunk's output projection
        return self.ps_o.tile([P, CW], F32, tag="oproj", name="ps_o")

    def big8(self):
        return self.sb.tile([P, DT, CW], F32, tag="big8", bufs=3, name="big8")

    def b4(self, dt=BF):
        return self.sb.tile([P, DT, CW], dt, tag="b4", bufs=4, name="b4")


def _layernorm_cw(tc, po, consts, x_chunk, out_f, out_b, post_bias=None):
    """LayerNorm over d_model for one [P, DT, CW] f32 chunk (two CH halves).

    The stats of both halves are reduced first so Ln/Exp run once each on a
    [1, CW] vector — one act-table round-trip per CW chunk instead of two.
    out_b: optional low-precision copy (taken BEFORE post_bias); post_bias:
    optional [P, DT] f32 bias folded into out_f only (residual-stream bias of
    the NEXT block's output projection).
    """
    nc = tc.nc
    mu = po.sb.tile([1, CW], F32, tag="ln_mu")
    msq = po.sb.tile([1, CW], F32, tag="ln_msq")
    var = po.sb.tile([1, CW], F32, tag="ln_var")
    rstd = po.sb.tile([1, CW], F32, tag="ln_rstd")
    for lh in range(CW // CH):
        l0 = lh * CH
        cx = po.sb.tile([P, DT, CH], BF, tag="b4", bufs=4, name="lncx")
        sq = po.sb.tile([P, DT, CH], BF, tag="b4", bufs=4, name="lnsq")
        xs = x_chunk[:, :, l0 : l0 + CH]
        nc.vector.tensor_copy(cx[:], xs)
        nc.vector.tensor_tensor(sq[:], xs, xs, OP.mult)
        pstat = po.ps_gen.tile([P, 2 * CH], F32, tag="gen")
        for kt in range(DT):
            nc.tensor.matmul(
                pstat[0:1, 0:CH], consts.ones_col[:], cx[:, kt, :],
                start=(kt == 0), stop=(kt == DT - 1), tile_position=(0, 0),
                skip_group_check=True,
            )
            nc.tensor.matmul(
                pstat[32:33, 0:CH], consts.ones_col[:], sq[:, kt, :],
                start=(kt == 0), stop=(kt == DT - 1), tile_position=(0, 32),
                skip_group_check=True,
            )
        nc.scalar.mul(mu[:, l0 : l0 + CH], pstat[0:1, 0:CH], 1.0 / D)
        nc.scalar.mul(msq[:, l0 : l0 + CH], pstat[32:33, 0:CH], 1.0 / D)
    nc.vector.tensor_tensor(var[:], mu[:], mu[:], OP.mult)
    nc.vector.tensor_sub(var[:], msq[:], var[:])
    nc.scalar.activation(var[:], var[:], AF.Ln, bias=consts.eps[0:1, :])
    nc.scalar.activation(rstd[:], var[:], AF.Exp, scale=-0.5)
    for lh in range(CW // CH):
        l0 = lh * CH
        pb = po.o_ps()
        nc.tensor.matmul(
            pb[:, 0:CH], consts.ones_row_f[:], mu[:, l0 : l0 + CH],
            start=True, stop=False,
        )
        nc.tensor.matmul(
            pb[:, CH : 2 * CH], consts.ones_row_f[:], rstd[:, l0 : l0 + CH],
            start=False, stop=True,
        )
        xs = x_chunk[:, :, l0 : l0 + CH]
        of = out_f[:, :, l0 : l0 + CH]
        mu_b = pb[:, 0:CH].rearrange("p (o n) -> p o n", o=1).to_broadcast((P, DT, CH))
        rs_b = pb[:, CH : 2 * CH].rearrange("p (o n) -> p o n", o=1).to_broadcast((P, DT, CH))
        nc.vector.tensor_tensor(of, xs, mu_b, OP.subtract)
        nc.vector.tensor_tensor(of, of, rs_b, OP.mult)
        if out_b is not None:
            nc.vector.tensor_copy(out_b[:, :, l0 : l0 + CH], of)
        if post_bias is not None:
            pb_b = post_bias.rearrange("p (t o) -> p t o", o=1).to_broadcast((P, DT, CH))
            nc.vector.tensor_tensor(of, of, pb_b, OP.add)


def _attention_chunk(tc, po, consts, KT, Vaug, qt_c, attn_c):
    """One query chunk (CW columns) of MHA in transposed layout, per head.

    KT: [P, DT, S] bf16; Vaug: [P, SKT, H, DK+1] (natural V per head with a
    ones column appended -> PV matmuls emit the softmax denominator in row 64);
    qt_c: [P, DT, CW] bf16 (pre-scaled by 1/8); attn_c: [P, DT, CW] out.
    """
    nc = tc.nc
    e_dt = F8 if FP8_PV else BF
    for h in range(H):
        hp, prow = h // 2, (h % 2) * DK
        comb = po.ps_pv.tile([P, CW], F32, tag="pv", name="comb")
        for sp in range(SKT // 2):  # pairs of key tiles
            k0 = 2 * sp
            ps_s = po.ps_big.tile([P, 2 * CW], F32, tag="scores", name="ps_s")
            for qi in range(2):
                # each half is a full PSUM bank: start zeroes its own region
                nc.tensor.matmul(
                    ps_s[:, qi * CW : (qi + 1) * CW],
                    KT[prow : prow + DK, hp, _t(k0 + qi)],
                    qt_c[prow : prow + DK, hp, :],
                    start=True, stop=True,
                )
            e = po.sb.tile([P, 2 * CW], e_dt, tag="exp", bufs=3, name="e")
            nc.scalar.activation(e[:], ps_s[:], AF.Exp)
            if FP8_PV:
                nc.tensor.matmul(
                    comb[0 : DK + 1, :],
                    Vaug[:, k0 : k0 + 2, h, :],
                    e[:].rearrange("p (two n) -> p two n", two=2),
                    start=(sp == 0), stop=(sp == SKT // 2 - 1),
                    perf_mode=DR,
                )
            else:
                for qi in range(2):
                    nc.tensor.matmul(
                        comb[0 : DK + 1, :],
                        Vaug[:, k0 + qi, h, :],
                        e[:, qi * CW : (qi + 1) * CW],
                        start=(sp == 0 and qi == 0),
                        stop=(sp == SKT // 2 - 1 and qi == 1),
                    )
        # normalize: the rowsum sits in row DK of the accumulator
        rf0 = po.sb.tile([1, CW], F32, tag="rf0", bufs=2, name="rf0")
        nc.vector.reciprocal(rf0[:], comb[DK : DK + 1, :])
        rfb = po.sb.tile([1, CW], BF, tag="rfb", bufs=2, name="rfb")
        nc.vector.tensor_copy(rfb[:], rf0[:])
        ps_r = po.ps_gen.tile([P, CW], F32, tag="gen", name="ps_r")
        nc.tensor.matmul(
            ps_r[0:DK, :], consts.ones_row_b[:, 0:DK], rfb[:], start=True, stop=True
        )
        rbc = po.sb.tile([DK, CW], BF, tag="rbc", bufs=2, name="rbc")
        nc.vector.tensor_copy(rbc[:], ps_r[0:DK, :])
        nc.vector.tensor_tensor(
            attn_c[prow : prow + DK, hp, :], comb[0:DK, :], rbc[:], OP.mult
        )


def build_program():
    nc = bacc.Bacc("TRN2", target_bir_lowering=False, debug=False, num_devices=8)

    act_dt = F8 if FP8_PROJ else BF
    ffn_dt = F8 if FP8_FFN else BF

    def din(name, shape, dt=BF):
        return nc.dram_tensor(name, list(shape), dt, kind="ExternalInput").ap()

    tgtT = din("tgtT", (D, S), act_dt)
    tgtLocT = din("tgtLocT", (D, SL), F32)
    srcT = din("srcT", (D, S), act_dt)
    w = {}
    for pre in ("sa", "ca"):
        for nm in ("wq", "wk", "wv", "wo"):
            w[f"{pre}_{nm}"] = din(f"{pre}_{nm}", (D, D), act_dt)
        w[f"{pre}_bqT"] = din(f"{pre}_bqT", (P, DT), F32)
        w[f"{pre}_bkT"] = din(f"{pre}_bkT", (P, DT), F32)
        w[f"{pre}_bvB"] = din(f"{pre}_bvB", (P, D), BF)
    w["ca_boT"] = din("ca_boT", (P, DT), F32)
    w["ff_w1"] = din("ff_w1", (D, DFF), ffn_dt)
    w["ff_w2"] = din("ff_w2", (DFF, D), ffn_dt)
    w["ff_b1T"] = din("ff_b1T", (P, FT), F32)
    w["ff_b2T"] = din("ff_b2T", (P, DT), F32)

    outT = nc.dram_tensor("outT", [D, SL], F32, kind="ExternalOutput").ap()
    x1f = nc.dram_tensor("x1f", [D, SL], F32).ap()
    x1b = nc.dram_tensor("x1b", [D, SL], act_dt).ap()
    x2f = nc.dram_tensor("x2f", [D, SL], F32).ap()

    def r3(ap):  # [(t p), s] dram -> [p, t, s]
        return ap.rearrange("(t p) s -> p t s", p=P)

    import contextlib

    reps = int(os.environ.get("KERNEL_REPS", "1"))
    with tile.TileContext(nc) as tc, contextlib.ExitStack() as ctx:
        po = Pools(tc, ctx)
        consts = _Consts(tc, po.sb)

        def load_w_block(dram_ap, t_n, cols, dt=BF):
            t_ = po.sb.tile([P, t_n, 1024], dt, tag="w", bufs=3, name="wblk")[:, :, : cols.stop - cols.start]
            nc.sync.dma_start(t_[:], r3(dram_ap)[:, :t_n, cols])
            return t_

        def load_bias(name, n):
            t_ = po.sb.tile([P, FT], F32, tag=f"b_{name}", name=f"b_{name}")[:, :n]
            nc.sync.dma_start(t_[:], w[name][:, :n])
            return t_

        def proj_T(w_sb, rhs_fn, evict_fn, n_cols, out_tiles=DT, cw=CW, dr=False):
            for t_out in range(out_tiles):
                for c0 in range(0, n_cols, cw):
                    pt = po.proj_ps(cw)
                    if dr:
                        for kt in range(0, DT, 2):
                            nc.tensor.matmul(
                                pt[:], w_sb[:, kt : kt + 2, _t(t_out)],
                                rhs_fn(kt, c0),
                                start=(kt == 0), stop=(kt == DT - 2),
                                perf_mode=DR,
                            )
                    else:
                        for kt in range(DT):
                            nc.tensor.matmul(
                                pt[:], w_sb[:, kt, _t(t_out)], rhs_fn(kt, c0),
                                start=(kt == 0), stop=(kt == DT - 1),
                            )
                    evict_fn(t_out, c0, pt)

        def attn_phase(pre, kv_srcT, q_loader, resid_f, x_out_f, x_out_b,
                       x_out_b_sb=None, post_bias=None, qw=CH):
            KT = po.sb.tile([P, DT, S], BF, tag="KT")
            v_dt = F8 if FP8_PV else BF
            Vaug = po.sb.tile([P, SKT, H, DK + 1], v_dt, tag="Vn")
            nc.vector.memset(Vaug[:, :, :, DK : DK + 1], 1.0)
            wk = load_w_block(w[f"{pre}_wk"], DT, slice(0, D), dt=act_dt)
            bkT = load_bias(f"{pre}_bkT", DT)
            wsc = 1.0 / WS1 if FP8_PROJ else None

            KW = 512

            def k_evict(t, c0, pt):
                if FP8_PROJ:
                    nc.vector.tensor_scalar(
                        KT[:, t, c0 : c0 + KW], pt[:], wsc,
                        bkT[:, t : t + 1], OP.mult, OP.add,
                    )
                else:
                    nc.vector.tensor_scalar_add(
                        KT[:, t, c0 : c0 + KW], pt[:], bkT[:, t : t + 1]
                    )

            proj_T(
                wk,
                lambda kt, c0: (
                    kv_srcT[:, kt : kt + 2, c0 : c0 + KW] if FP8_PROJ
                    else kv_srcT[:, kt, c0 : c0 + KW]
                ),
                k_evict, S, cw=KW, dr=FP8_PROJ,
            )
            wv = load_w_block(w[f"{pre}_wv"], DT, slice(0, D), dt=act_dt)
            bvB = po.sb.tile([P, D], BF, tag="bvB", bufs=1)
            nc.sync.dma_start(bvB[:], w[f"{pre}_bvB"][:])
            VW = 512
            HPC = VW // DK  # heads per column chunk
            for skt in range(SKT):
                for dc in range(D // VW):
                    pt = po.proj_ps(VW)
                    if FP8_PROJ:
                        for kt in range(0, DT, 2):
                            nc.tensor.matmul(
                                pt[:], kv_srcT[:, kt : kt + 2, _t(skt)],
                                wv[:, kt : kt + 2, dc * VW : (dc + 1) * VW],
                                start=(kt == 0), stop=(kt == DT - 2),
                                perf_mode=DR,
                            )
                    else:
                        for kt in range(DT):
                            nc.tensor.matmul(
                                pt[:], kv_srcT[:, kt, _t(skt)],
                                wv[:, kt, dc * VW : (dc + 1) * VW],
                                start=(kt == 0), stop=(kt == DT - 1),
                            )
                    if FP8_PROJ:
                        nc.vector.scalar_tensor_tensor(
                            Vaug[:, skt, dc * HPC : (dc + 1) * HPC, 0:DK],
                            pt[:].rearrange("p (a b) -> p a b", a=HPC),
                            wsc,
                            bvB[:, dc * VW : (dc + 1) * VW].rearrange(
                                "p (a b) -> p a b", a=HPC
                            ),
                            OP.mult, OP.add,
                        )
                    else:
                        nc.vector.tensor_tensor(
                            Vaug[:, skt, dc * HPC : (dc + 1) * HPC, 0:DK],
                            pt[:].rearrange("p (a b) -> p a b", a=HPC),
                            bvB[:, dc * VW : (dc + 1) * VW].rearrange(
                                "p (a b) -> p a b", a=HPC
                            ),
                            OP.add,
                        )
            wq = load_w_block(w[f"{pre}_wq"], DT, slice(0, D), dt=act_dt)
            bqT = load_bias(f"{pre}_bqT", DT)  # pre-scaled by 1/8 on host
            wo = load_w_block(w[f"{pre}_wo"], DT, slice(0, D), dt=act_dt)
            qsc = 0.125 / WS1 if FP8_PROJ else 0.125

            def q_evict(t, c0, pt, dest):
                nc.vector.tensor_scalar(
                    dest, pt[:], qsc, bqT[:, t : t + 1], OP.mult, OP.add
                )

            # project Q for ALL chunks up-front (frees kv/q sources early and
            # lets the attention chunks pipeline back-to-back)
            qt_all = po.sb.tile([P, DT, SL], BF, tag="qtA", name="qt_all")
            for b0 in range(0, SL, qw):
                q_src = q_loader(b0)
                proj_T(
                    wq,
                    lambda kt, c0, q_src=q_src: q_src(kt, c0),
                    lambda t, c0, pt, b0=b0: q_evict(
                        t, c0, pt, qt_all[:, t, b0 + c0 : b0 + c0 + 512]
                    ),
                    qw, cw=512, dr=FP8_PROJ,
                )
            for c in range(NCW):
                c0 = c * CW
                attn_c = po.b4(act_dt)
                _attention_chunk(
                    tc, po, consts, KT, Vaug, qt_all[:, :, c0 : c0 + CW], attn_c
                )
                x_chunk = po.big8()
                rt3 = resid_f(c0)
                for t_out in range(DT):
                    pt = (
                        po.o_ps() if t_out % 2 == 0
                        else po.ps_gen.tile([P, CW], F32, tag="gen", name="ps_g")
                    )
                    if FP8_PROJ:
                        for kt in range(0, DT, 2):
                            nc.tensor.matmul(
                                pt[:], wo[:, kt : kt + 2, _t(t_out)],
                                attn_c[:, kt : kt + 2, :],
                                start=(kt == 0), stop=(kt == DT - 2),
                                perf_mode=DR,
                            )
                        nc.vector.scalar_tensor_tensor(
                            x_chunk[:, t_out, :], pt[:], wsc,
                            rt3[:, t_out, :], OP.mult, OP.add,
                        )
                    else:
                        for kt in range(DT):
                            nc.tensor.matmul(
                                pt[:], wo[:, kt, _t(t_out)], attn_c[:, kt, :],
                                start=(kt == 0), stop=(kt == DT - 1),
                            )
                        nc.vector.tensor_tensor(
                            x_chunk[:, t_out, :], pt[:], rt3[:, t_out, :], OP.add
                        )
                xnf = po.big8()
                xnb = (
                    x_out_b_sb[:, :, c0 : c0 + CW]
                    if x_out_b_sb is not None
                    else po.b4(act_dt)
                )
                _layernorm_cw(tc, po, consts, x_chunk, xnf, xnb, post_bias)
                nc.sync.dma_start(r3(x_out_f)[:, :, c0 : c0 + CW], xnf[:])
                if x_out_b_sb is None:
                    nc.sync.dma_start(r3(x_out_b)[:, :, c0 : c0 + CW], xnb[:])

        phases = os.environ.get("KERNEL_PHASES", "abc")
        for _rep in range(reps):
            # ---- Phase A: self-attention on tgt ----
            tgtT_sb = po.sb.tile([P, DT, S], act_dt, tag="actT", name="tgtT_sb")
            for s0 in range(0, S, 512):  # column-split so K-proj starts early
                nc.sync.dma_start(
                    tgtT_sb[:, :, s0 : s0 + 512], r3(tgtT)[:, :, s0 : s0 + 512]
                )

            def tgt_resid(c0):
                # tgtLocT has sa_bo pre-added on the host
                rt = po.sb.tile([P, DT, CW], F32, tag="resid", bufs=1, name="resid")
                nc.sync.dma_start(rt[:], r3(tgtLocT)[:, :, c0 : c0 + CW])
                return rt[:]

            ca_boT = load_bias("ca_boT", DT)
            attn_phase(
                "sa", tgtT_sb,
                lambda b0: (lambda kt, c0: (
                    tgtT_sb[:, kt : kt + 2, b0 + c0 : b0 + c0 + 512] if FP8_PROJ
                    else tgtT_sb[:, kt, b0 + c0 : b0 + c0 + 512]
                )),
                tgt_resid, x1f, x1b, post_bias=ca_boT, qw=SL,
            )

            if "b" not in phases:
                continue
            # ---- Phase B: cross-attention ----
            srcT_sb = po.sb.tile([P, DT, S], act_dt, tag="actT", name="srcT_sb")
            for s0 in range(0, S, 512):
                nc.sync.dma_start(
                    srcT_sb[:, :, s0 : s0 + 512], r3(srcT)[:, :, s0 : s0 + 512]
                )

            def x1_qsrc(b0):
                qt = po.sb.tile([P, DT, SL], act_dt, tag="big8", bufs=3, name="qsrc")
                nc.sync.dma_start(qt[:], r3(x1b)[:, :, b0 : b0 + SL])
                return lambda kt, c0: (
                    qt[:, kt : kt + 2, c0 : c0 + 512] if FP8_PROJ
                    else qt[:, kt, c0 : c0 + 512]
                )

            def x1_resid(c0):
                # x1f has ca_bo folded in by phase A's LayerNorm post-bias
                rt = po.sb.tile([P, DT, CW], F32, tag="resid", bufs=1, name="resid")
                nc.sync.dma_start(rt[:], r3(x1f)[:, :, c0 : c0 + CW])
                return rt[:]

            # LN2's low-precision output stays in SBUF for phase C (actT slot
            # is free once the ca K/V projections have consumed srcT_sb)
            ff_b2T = load_bias("ff_b2T", DT)
            x2n_all = po.sb.tile([P, DT, SL], ffn_dt, tag="actT", name="x2n_all")
            attn_phase("ca", srcT_sb, x1_qsrc, x1_resid, x2f, None,
                       x_out_b_sb=x2n_all, post_bias=ff_b2T, qw=SL)

            if "c" not in phases:
                continue
            # ---- Phase C: FFN, d_ff quarters OUTER so each weight block is
            # loaded once; partial sums for all chunks accumulate in SBUF ----
            b1T = load_bias("ff_b1T", FT)
            QF = 1024 // P  # ff-tiles per quarter
            acc_all = po.sb.tile([P, DT, SL], F32, tag="KT", name="acc_all")
            w2sc = 1.0 / WS2 if FP8_FFN else None
            for qtr in range(4):
                w1q = load_w_block(w["ff_w1"], DT, slice(qtr * 1024, (qtr + 1) * 1024), dt=ffn_dt)
                w2q = po.sb.tile([P, QF, D], ffn_dt, tag="w", bufs=3, name="w2q")
                nc.sync.dma_start(
                    w2q[:], r3(w["ff_w2"])[:, qtr * QF : (qtr + 1) * QF, :]
                )
                for c in range(NCW):
                    c0 = c * CW
                    hq = po.sb.tile([P, QF, CW], ffn_dt, tag="b4", bufs=4, name="hq")
                    for fo in range(QF):
                        ft = qtr * QF + fo
                        pt = po.proj_ps()
                        if FP8_FFN:
                            for kt in range(0, DT, 2):
                                nc.tensor.matmul(
                                    pt[:], w1q[:, kt : kt + 2, _t(fo)],
                                    x2n_all[:, kt : kt + 2, c0 : c0 + CW],
                                    start=(kt == 0), stop=(kt == DT - 2),
                                    perf_mode=DR,
                                )
                            nc.scalar.activation(
                                hq[:, fo, :], pt[:], AF.Relu,
                                bias=b1T[:, ft : ft + 1], scale=1.0 / WS1,
                            )
                        else:
                            for kt in range(DT):
                                nc.tensor.matmul(
                                    pt[:], w1q[:, kt, _t(fo)],
                                    x2n_all[:, kt, c0 : c0 + CW],
                                    start=(kt == 0), stop=(kt == DT - 1),
                                )
                            nc.scalar.activation(
                                hq[:, fo, :], pt[:], AF.Relu, bias=b1T[:, ft : ft + 1]
                            )
                    for t_out in range(DT):
                        pt = po.proj_ps()
                        if FP8_FFN:
                            for fo in range(0, QF, 2):
                                nc.tensor.matmul(
                                    pt[:], w2q[:, fo : fo + 2, _t(t_out)],
                                    hq[:, fo : fo + 2, :],
                                    start=(fo == 0), stop=(fo == QF - 2),
                                    perf_mode=DR,
                                )
                            if qtr == 0:
                                nc.vector.tensor_scalar_mul(
                                    acc_all[:, t_out, c0 : c0 + CW], pt[:], w2sc
                                )
                            else:
                                nc.vector.scalar_tensor_tensor(
                                    acc_all[:, t_out, c0 : c0 + CW], pt[:], w2sc,
                                    acc_all[:, t_out, c0 : c0 + CW], OP.mult, OP.add,
                                )
                        else:
                            for fo in range(QF):
                                nc.tensor.matmul(
                                    pt[:], w2q[:, fo, _t(t_out)], hq[:, fo, :],
                                    start=(fo == 0), stop=(fo == QF - 1),
                                )
                            if qtr == 0:
                                nc.vector.tensor_copy(acc_all[:, t_out, c0 : c0 + CW], pt[:])
                            else:
                                nc.vector.tensor_tensor(
                                    acc_all[:, t_out, c0 : c0 + CW],
                                    acc_all[:, t_out, c0 : c0 + CW], pt[:], OP.add,
                                )
                    if qtr == 3:
                        # chunk c is fully accumulated: residual + LN now, so
                        # the LN chains overlap chunk c+1's matmuls
                        x3_chunk = po.big8()
                        # x2f has ff_b2 folded in by phase B's LN post-bias
                        rt3 = po.sb.tile([P, DT, CW], F32, tag="resid", bufs=1, name="resid")
                        nc.sync.dma_start(rt3[:], r3(x2f)[:, :, c0 : c0 + CW])
                        nc.vector.tensor_tensor(
                            x3_chunk[:], acc_all[:, :, c0 : c0 + CW], rt3[:], OP.add,
                        )
                        out_f = po.big8()
                        _layernorm_cw(tc, po, consts, x3_chunk, out_f, None)
                        nc.sync.dma_start(r3(outT)[:, :, c0 : c0 + CW], out_f[:])

    nc.compile()
    return nc


_NC_CACHE = {}


def _get_nc():
    if "nc" not in _NC_CACHE:
        _NC_CACHE["nc"] = build_program()
    return _NC_CACHE["nc"]


def make_in_maps(inputs):
    tgt = np.asarray(inputs["tgt"], np.float32)
    src = np.asarray(inputs["src"], np.float32)
    act_np = F8_NP if FP8_PROJ else BF_NP
    ffn_np = F8_NP if FP8_FFN else BF_NP
    wsc = WS1 if FP8_PROJ else 1.0

    shared = {}
    for pre in ("sa", "ca"):
        for nm in ("wq", "wk", "wv", "wo"):
            shared[f"{pre}_{nm}"] = np.ascontiguousarray(
                (np.asarray(inputs[f"{pre}_{nm}"], np.float32) * wsc).astype(act_np)
            )
        bq = np.asarray(inputs[f"{pre}_bq"], np.float32) * 0.125
        shared[f"{pre}_bqT"] = np.ascontiguousarray(bq.reshape(DT, P).T)
        shared[f"{pre}_bkT"] = np.ascontiguousarray(
            np.asarray(inputs[f"{pre}_bk"], np.float32).reshape(DT, P).T
        )
        shared[f"{pre}_bvB"] = np.ascontiguousarray(
            np.broadcast_to(np.asarray(inputs[f"{pre}_bv"]), (P, D)).astype(BF_NP)
        )
    shared["ca_boT"] = np.ascontiguousarray(
        np.asarray(inputs["ca_bo"], np.float32).reshape(DT, P).T
    )
    w2sc = WS2 if FP8_FFN else 1.0
    shared["ff_w1"] = np.ascontiguousarray(
        (np.asarray(inputs["ff_w1"], np.float32) * (WS1 if FP8_FFN else 1.0)).astype(ffn_np)
    )
    shared["ff_w2"] = np.ascontiguousarray(
        (np.asarray(inputs["ff_w2"], np.float32) * w2sc).astype(ffn_np)
    )
    shared["ff_b1T"] = np.ascontiguousarray(
        np.asarray(inputs["ff_b1"], np.float32).reshape(FT, P).T
    )
    shared["ff_b2T"] = np.ascontiguousarray(
        np.asarray(inputs["ff_b2"], np.float32).reshape(DT, P).T
    )
    sa_bo = np.asarray(inputs["sa_bo"], np.float32)

    in_maps = []
    for core in range(8):
        b, q = core // 2, core % 2
        tT = np.ascontiguousarray(tgt[b].T)  # [D, S] f32
        if q == 1:
            tT = np.concatenate([tT[:, SL:], tT[:, :SL]], axis=1)
        m = dict(shared)
        m["tgtT"] = np.ascontiguousarray(tT.astype(act_np))
        m["tgtLocT"] = np.ascontiguousarray(tT[:, :SL] + sa_bo[:, None])
        m["srcT"] = np.ascontiguousarray(src[b].T.astype(act_np))
        in_maps.append(m)
    return in_maps


def assemble_output(results):
    out = np.empty((B, S, D), np.float32)
    for core in range(8):
        b, q = core // 2, core % 2
        out[b, q * SL : (q + 1) * SL, :] = results[core]["outT"].T
    return out


def kernel(**inputs):
    nc = _get_nc()
    in_maps = make_in_maps(inputs)
    res = run_bass_kernel_spmd(nc, in_maps, list(range(8)))
    return assemble_output(res.results)


if __name__ == "__main__":
    nc = build_program()
    print("program built + compiled OK")
